# revision 2
# baseline (speedup 1.0000x reference)
"""Trainium2 kernel for: out = tanh(x @ scatter_nd(nonzero_ind, kernel_vector, (20000, 4096)) + bias).

Strategy (8 NeuronCores):
  - Host builds the dense (20000, 4096) weight matrix from the COO triples and
    pre-transposes x shards, zero-padded to 2 contraction shards of 10112 rows
    (79 k-tiles each; 224 rows total padding vs 480 for a 4-way split).
  - Shard: contraction K x2, batch x4  ->  core c = (batch quarter h, k half q)
    computes partial[h,q] = x[h*512:(h+1)*512, qK] @ W[qK, :]  (512 x 4096).
  - On device: the transposed x shard (10112 x 512) lives SBUF-resident as 79
    [128 x 512] fp16 tiles (stationary matmul operand); W streams through once
    in fp16 as contiguous pre-arranged [128 x 512] tiles; matmuls (fp32 PSUM
    accumulation) run across all 79 k-tiles into 4 PSUM banks (bank b = batch
    tile b), double-buffered across the 8 unit blocks so unit-block boundaries
    never stall the PE. The x-resident loads interleave with the first unit
    block's k-loop so the PE starts ~1us after kernel entry.
  - Host sums the 2 K-partials per batch quarter, adds bias, applies tanh.

Per core: 2528 matmuls ([128x128] stationary x [128x512] moving, ~216.8 ns
steady) ~= 548 us PE time; DMA 104 MB/core well under compute.
"""

import numpy as np

P = 128
B, K, U = 2048, 20000, 4096
KSPLIT, HSPLIT = 2, 4
KT = 79                  # k-tiles per shard
KPAD = KT * P            # 10112 rows per K-shard (2 * 10112 = 20224 >= 20000)
B_SH = B // HSPLIT       # 512 batch rows per core
NBT = B_SH // P          # 4 batch tiles -> 4 PSUM banks per phase
UBLK = 512               # moving free dim per matmul
NUB = U // UBLK          # 8 unit blocks

TRACE = False            # set by test harness for profiled runs
LAST_RESULT = None       # BassKernelResults of the last run (for the harness)

_NC_CACHE = {}


def _build_nc():
    from concourse import bacc
    import concourse.mybir as mybir
    import concourse.tile as tile

    f32 = mybir.dt.float32
    f16 = mybir.dt.float16

    nc = bacc.Bacc("TRN2", target_bir_lowering=False, debug=False)
    xt_d = nc.dram_tensor("xt_sh", [KT, P, B_SH], f16, kind="ExternalInput").ap()
    w_d = nc.dram_tensor("w_sh", [NUB, KT, P, UBLK], f16, kind="ExternalInput").ap()
    o_d = nc.dram_tensor("out_p", [B_SH, U], f32, kind="ExternalOutput").ap()

    with tile.TileContext(nc) as tc:
        with (
            tc.tile_pool(name="resid", bufs=1) as respool,
            tc.tile_pool(name="wpool", bufs=8) as wpool,
            tc.tile_pool(name="stage", bufs=8) as spool,
            tc.tile_pool(name="mpsum", bufs=2, space="PSUM") as mpsum,
        ):
            # Resident transposed-x tiles; DMAs are emitted interleaved with
            # the first unit block's k-loop so the PE pipeline fills
            # immediately instead of waiting behind the whole 10MB x load.
            xt = [
                respool.tile([P, B_SH], f16, tag=f"xt{kt}", name=f"xt{kt}")
                for kt in range(KT)
            ]

            # out[b, u] += xT[k, b].T @ W[k, u], accumulated over all k-tiles
            # in PSUM bank bi (double-buffered across unit blocks), streamed
            # over 512-wide unit blocks.
            for ub in range(NUB):
                psums = [
                    mpsum.tile([P, UBLK], f32, tag=f"ps{bi}", name=f"ps{bi}")
                    for bi in range(NBT)
                ]
                for kt in range(KT):
                    if ub == 0:
                        nc.sync.dma_start(xt[kt][:], xt_d[kt])
                    wt = wpool.tile([P, UBLK], f16, tag="wt", name="wt")
                    nc.sync.dma_start(wt[:], w_d[ub, kt])
                    for bi in range(NBT):
                        nc.tensor.matmul(
                            psums[bi][:],
                            xt[kt][:, bi * P:(bi + 1) * P],
                            wt[:],
                            start=(kt == 0),
                            stop=(kt == KT - 1),
                        )
                for bi in range(NBT):
                    st = spool.tile([P, UBLK], f32, tag="st", name="st")
                    nc.vector.tensor_copy(st[:], psums[bi][:])
                    nc.sync.dma_start(
                        o_d[bi * P:(bi + 1) * P, ub * UBLK:(ub + 1) * UBLK],
                        st[:],
                    )

    nc.compile()
    return nc


def _get_nc():
    if "nc" not in _NC_CACHE:
        _NC_CACHE["nc"] = _build_nc()
    return _NC_CACHE["nc"]


def kernel(x, kernel_vector, bias, nonzero_ind):
    global LAST_RESULT
    from concourse.bass_utils import run_bass_kernel_spmd

    x = np.asarray(x, dtype=np.float32)
    kernel_vector = np.asarray(kernel_vector, dtype=np.float32)
    bias = np.asarray(bias, dtype=np.float32)
    nonzero_ind = np.asarray(nonzero_ind)

    nc = _get_nc()

    # Host scatter: dense weights, rows padded to KSPLIT * KPAD, then
    # pre-arranged per K-shard as contiguous [NUB, KT, 128, 512] tiles.
    rows = nonzero_ind[:, 0].astype(np.int64)
    cols = nonzero_ind[:, 1].astype(np.int64)
    w_full = np.zeros(KSPLIT * KPAD * U, np.float32)
    np.add.at(w_full, rows * U + cols, kernel_vector)
    w_full = w_full.reshape(KSPLIT * KPAD, U).astype(np.float16)
    # [KSPLIT*KPAD, U] -> per shard [KT, 128, NUB, 512] -> [NUB, KT, 128, 512]
    w_sh = [
        np.ascontiguousarray(
            w_full[q * KPAD:(q + 1) * KPAD]
            .reshape(KT, P, NUB, UBLK)
            .transpose(2, 0, 1, 3)
        )
        for q in range(KSPLIT)
    ]
    x16 = x.astype(np.float16)

    in_maps = []
    for c in range(8):
        h, q = divmod(c, KSPLIT)
        k0 = q * KPAD
        k1 = min(K, k0 + KPAD)
        xs = np.zeros((KPAD, B_SH), np.float16)
        xs[: k1 - k0] = x16[h * B_SH:(h + 1) * B_SH, k0:k1].T
        in_maps.append(
            {"xt_sh": xs.reshape(KT, P, B_SH), "w_sh": w_sh[q]}
        )

    kwargs = {}
    if TRACE:
        kwargs = dict(trace=True, trace_cores=list(range(8)))
    res = run_bass_kernel_spmd(nc, in_maps, core_ids=list(range(8)), **kwargs)
    LAST_RESULT = res

    out = np.empty((B, U), np.float32)
    for h in range(HSPLIT):
        acc = res.results[h * KSPLIT]["out_p"].copy()
        for q in range(1, KSPLIT):
            acc += res.results[h * KSPLIT + q]["out_p"]
        acc += bias[None, :]
        np.tanh(acc, out=acc)
        out[h * B_SH:(h + 1) * B_SH] = acc
    return out


# revision 3
# speedup vs baseline: 1.0253x; 1.0253x over previous
"""Trainium2 kernel for: out = tanh(x @ scatter_nd(nonzero_ind, kernel_vector, (20000, 4096)) + bias).

Strategy (8 NeuronCores):
  - Host builds the dense (20000, 4096) weight matrix from the COO triples and
    pre-transposes x shards, zero-padded to 2 contraction shards of 10112 rows
    (79 k-tiles each; 224 rows total padding vs 480 for a 4-way split).
  - Shard: contraction K x2, batch x4  ->  core c = (batch quarter h, k half q)
    computes partial[h,q] = x[h*512:(h+1)*512, qK] @ W[qK, :]  (512 x 4096).
  - On device: the transposed x shard (10112 x 512) lives SBUF-resident as 79
    [128 x 512] fp16 tiles (stationary matmul operand); W streams through once
    in fp16 as contiguous pre-paired [128 x 1024] tiles (two k-tiles per DMA).
    DMA triggers are spread across both hardware DGE queues (sync + scalar,
    ~160 GB/s each) because one queue can neither issue triggers fast enough
    (~605 ns each) nor stream xt+W concurrently during the first unit block
    (~300 GB/s demand).  Output DMAs ride the gpsimd SWDGE queue so unit-block
    boundaries never preempt the W stream.  Matmuls accumulate over all 79
    k-tiles into 4 PSUM banks (bank b = batch tile b), double-buffered across
    the 8 unit blocks so block boundaries never stall the PE.
  - Host sums the 2 K-partials per batch quarter, adds bias, applies tanh.

Per core: 2528 matmuls ([128x128] stationary x [128x512] moving, ~216.8 ns
steady) ~= 548 us PE time.
"""

import numpy as np

P = 128
B, K, U = 2048, 20000, 4096
KSPLIT, HSPLIT = 2, 4
KT = 79                  # k-tiles per shard (last one partially zero-padded)
KT2 = 40                 # k-tile pairs per shard (last pair has 1 real tile)
KPAD = KT * P            # 10112 rows per K-shard (2 * 10112 = 20224 >= 20000)
B_SH = B // HSPLIT       # 512 batch rows per core
NBT = B_SH // P          # 4 batch tiles -> 4 PSUM banks per phase
UBLK = 512               # moving free dim per matmul
NUB = U // UBLK          # 8 unit blocks

TRACE = False            # set by test harness for profiled runs
LAST_RESULT = None       # BassKernelResults of the last run (for the harness)

_NC_CACHE = {}


def _build_nc():
    from concourse import bacc
    import concourse.mybir as mybir
    import concourse.tile as tile

    f32 = mybir.dt.float32
    f16 = mybir.dt.float16

    nc = bacc.Bacc("TRN2", target_bir_lowering=False, debug=False)
    xt_d = nc.dram_tensor("xt_sh", [KT, P, B_SH], f16, kind="ExternalInput").ap()
    # W pre-paired: [ub, ktp, p, j*512+c] = W[(2*ktp+j)*128 + p, ub*512 + c]
    w_d = nc.dram_tensor("w_sh", [NUB, KT2, P, 2 * UBLK], f16,
                         kind="ExternalInput").ap()
    o_d = nc.dram_tensor("out_p", [B_SH, U], f32, kind="ExternalOutput").ap()

    with tile.TileContext(nc) as tc:
        with (
            tc.tile_pool(name="resid", bufs=1) as respool,
            tc.tile_pool(name="wpool", bufs=5) as wpool,
            tc.tile_pool(name="stage", bufs=8) as spool,
            tc.tile_pool(name="mpsum", bufs=2, space="PSUM") as mpsum,
        ):
            # Resident transposed-x tiles; DMAs are emitted interleaved with
            # the first unit block's k-loop so the PE pipeline fills
            # immediately instead of waiting behind the whole 10MB x load.
            xt = [
                respool.tile([P, B_SH], f16, tag=f"xt{kt}", name=f"xt{kt}")
                for kt in range(KT)
            ]

            # out[b, u] += xT[k, b].T @ W[k, u], accumulated over all k-tiles
            # in PSUM bank bi (double-buffered across unit blocks), streamed
            # over 512-wide unit blocks.
            for ub in range(NUB):
                psums = [
                    mpsum.tile([P, UBLK], f32, tag=f"ps{bi}", name=f"ps{bi}")
                    for bi in range(NBT)
                ]
                for ktp in range(KT2):
                    weng = nc.sync if ktp % 2 == 0 else nc.scalar
                    wt = wpool.tile([P, 2 * UBLK], f16, tag="wt", name="wt")
                    if ktp < KT2 - 1:
                        weng.dma_start(wt[:], w_d[ub, ktp])
                        nj = 2
                    else:
                        weng.dma_start(wt[:, :UBLK], w_d[ub, ktp, :, :UBLK])
                        nj = 1
                    for j in range(nj):
                        kt = 2 * ktp + j
                        if ub == 0:
                            xeng = nc.scalar if kt % 2 == 0 else nc.sync
                            xeng.dma_start(xt[kt][:], xt_d[kt])
                        for bi in range(NBT):
                            nc.tensor.matmul(
                                psums[bi][:],
                                xt[kt][:, bi * P:(bi + 1) * P],
                                wt[:, j * UBLK:(j + 1) * UBLK],
                                start=(kt == 0),
                                stop=(kt == KT - 1),
                            )
                for bi in range(NBT):
                    st = spool.tile([P, UBLK], f32, tag="st", name="st")
                    nc.vector.tensor_copy(st[:], psums[bi][:])
                    if ub < NUB - 1:
                        oeng = nc.gpsimd
                    else:
                        oeng = (nc.gpsimd, nc.sync, nc.scalar, nc.gpsimd)[bi]
                    oeng.dma_start(
                        o_d[bi * P:(bi + 1) * P, ub * UBLK:(ub + 1) * UBLK],
                        st[:],
                    )

    nc.compile()
    return nc


def _get_nc():
    if "nc" not in _NC_CACHE:
        _NC_CACHE["nc"] = _build_nc()
    return _NC_CACHE["nc"]


def kernel(x, kernel_vector, bias, nonzero_ind):
    global LAST_RESULT
    from concourse.bass_utils import run_bass_kernel_spmd

    x = np.asarray(x, dtype=np.float32)
    kernel_vector = np.asarray(kernel_vector, dtype=np.float32)
    bias = np.asarray(bias, dtype=np.float32)
    nonzero_ind = np.asarray(nonzero_ind)

    nc = _get_nc()

    # Host scatter: dense weights, rows padded to 2*KT2*2*128, then
    # pre-arranged per K-shard as contiguous paired [NUB, KT2, 128, 1024]
    # tiles: [ub, ktp, p, j*512+c] = W[kshard + (2*ktp+j)*128 + p, ub*512+c].
    rows = nonzero_ind[:, 0].astype(np.int64)
    cols = nonzero_ind[:, 1].astype(np.int64)
    kp2 = KT2 * 2 * P    # 10240 rows per shard incl. pairing pad
    w_full = np.zeros(KSPLIT * kp2 * U, np.float32)
    idx = (rows // KPAD) * kp2 + (rows % KPAD)
    np.add.at(w_full, idx * U + cols, kernel_vector)
    w_full = w_full.reshape(KSPLIT, kp2, U).astype(np.float16)
    # [kp2, U] -> [KT2, 2, 128, NUB, 512] -> [NUB, KT2, 128, 2, 512]
    w_sh = [
        np.ascontiguousarray(
            w_full[q]
            .reshape(KT2, 2, P, NUB, UBLK)
            .transpose(3, 0, 2, 1, 4)
        ).reshape(NUB, KT2, P, 2 * UBLK)
        for q in range(KSPLIT)
    ]
    x16 = x.astype(np.float16)

    in_maps = []
    for c in range(8):
        h, q = divmod(c, KSPLIT)
        k0 = q * KPAD
        k1 = min(K, k0 + KPAD)
        xs = np.zeros((KPAD, B_SH), np.float16)
        xs[: k1 - k0] = x16[h * B_SH:(h + 1) * B_SH, k0:k1].T
        in_maps.append(
            {"xt_sh": xs.reshape(KT, P, B_SH), "w_sh": w_sh[q]}
        )

    kwargs = {}
    if TRACE:
        kwargs = dict(trace=True, trace_cores=list(range(8)))
    res = run_bass_kernel_spmd(nc, in_maps, core_ids=list(range(8)), **kwargs)
    LAST_RESULT = res

    out = np.empty((B, U), np.float32)
    for h in range(HSPLIT):
        acc = res.results[h * KSPLIT]["out_p"].copy()
        for q in range(1, KSPLIT):
            acc += res.results[h * KSPLIT + q]["out_p"]
        acc += bias[None, :]
        np.tanh(acc, out=acc)
        out[h * B_SH:(h + 1) * B_SH] = acc
    return out
